# revision 1
# baseline (speedup 1.0000x reference)
import sys, os
sys.path.insert(0, "/opt/trn_rl_repo")
import numpy as np
import ml_dtypes

import concourse.bass as bass
import concourse.tile as tile
import concourse.mybir as mybir
from concourse import bacc, bass_utils

B, D, H = 16384, 64, 256
NCORES = 8
BLOC = B // NCORES          # 2048 rows per core
BT = 512                    # b-chunk (free dim per matmul)
NCH = BLOC // BT            # 4 chunks
BF16 = mybir.dt.bfloat16
F32 = mybir.dt.float32
AF = mybir.ActivationFunctionType
ALU = mybir.AluOpType

_cache = {}


def _build():
    nc = bacc.Bacc("TRN2", target_bir_lowering=False, debug=False)
    dt = nc.dram_tensor
    # per-core tensors (x shard differs per core; weights replicated)
    xin = dt("xt32c", [4, 2, 16, BLOC], BF16, kind="ExternalInput")
    w1 = dt("w1c", [4, 2, 16, H], BF16, kind="ExternalInput")
    w2 = dt("w2sb", [128, D, 2, 2, 128], BF16, kind="ExternalInput")
    b2 = dt("b2t", [128, 2, D], F32, kind="ExternalInput")
    w3 = dt("w3d", [128, 2, D, D], BF16, kind="ExternalInput")
    b3 = dt("b3c", [D, 1], F32, kind="ExternalInput")
    wq = dt("wqt", [D, H], BF16, kind="ExternalInput")
    bq = dt("bqt", [128, 2], F32, kind="ExternalInput")
    wk = dt("wkt", [D, H], BF16, kind="ExternalInput")
    bk = dt("bkt", [128, 2], F32, kind="ExternalInput")
    ws = dt("wst", [128, 2, D], BF16, kind="ExternalInput")
    bs = dt("bsc", [D, 1], F32, kind="ExternalInput")
    wc1 = dt("wc1t", [D, H], BF16, kind="ExternalInput")
    bc1 = dt("bc1t", [128, 2], F32, kind="ExternalInput")
    wc2 = dt("wc2c", [128, 2], BF16, kind="ExternalInput")
    bc2 = dt("bc2a", [1, 1], F32, kind="ExternalInput")
    wg1 = dt("wg1r", [1, H], BF16, kind="ExternalInput")
    bg1 = dt("bg1t", [128, 2], F32, kind="ExternalInput")
    wg2 = dt("wg2c", [128, 2], BF16, kind="ExternalInput")
    bg2 = dt("bg2a", [1, 1], F32, kind="ExternalInput")
    ones = dt("ones64", [D, 1], F32, kind="ExternalInput")
    yout = dt("y", [NCH, BT], F32, kind="ExternalOutput")

    with tile.TileContext(nc) as tc:
        with (
            tc.tile_pool(name="w", bufs=1) as wp,
            tc.tile_pool(name="xp", bufs=2) as xp,
            tc.tile_pool(name="a1pool", bufs=6) as ap1,
            tc.tile_pool(name="a2pool", bufs=4) as ap2,
            tc.tile_pool(name="attn", bufs=1) as atp,
            tc.tile_pool(name="ps_a1", bufs=2, space="PSUM") as ps1,
            tc.tile_pool(name="ps_a2", bufs=2, space="PSUM") as ps2,
            tc.tile_pool(name="ps_h", bufs=1, space="PSUM") as psh,
            tc.tile_pool(name="ps_at", bufs=1, space="PSUM") as psa,
        ):
            # ---- load weights into SBUF (resident) ----
            w1s = wp.tile([128, 16, H], BF16)
            for g in range(4):
                nc.sync.dma_start(w1s[32 * g : 32 * g + 2], w1.ap()[g])
            b2s = wp.tile([128, 2, D], F32)
            nc.sync.dma_start(b2s[:], b2.ap())
            w3s = wp.tile([128, 2, D, D], BF16)
            nc.sync.dma_start(w3s[:], w3.ap())
            smalls = {}
            for name, t, shape, dtp in (
                ("b3", b3, [D, 1], F32), ("wq", wq, [D, H], BF16), ("bq", bq, [128, 2], F32),
                ("wk", wk, [D, H], BF16), ("bk", bk, [128, 2], F32), ("ws", ws, [128, 2, D], BF16),
                ("bs", bs, [D, 1], F32), ("wc1", wc1, [D, H], BF16), ("bc1", bc1, [128, 2], F32),
                ("wc2", wc2, [128, 2], BF16), ("bc2", bc2, [1, 1], F32), ("wg1", wg1, [1, H], BF16),
                ("bg1", bg1, [128, 2], F32), ("wg2", wg2, [128, 2], BF16), ("bg2", bg2, [1, 1], F32),
                ("ones", ones, [D, 1], F32),
            ):
                st = wp.tile(shape, dtp, tag="small_" + name)
                nc.sync.dma_start(st[:], t.ap())
                smalls[name] = st

            # W2 is the bulk of the preamble DMA (8.4 MB); queue it last so
            # chunk-0's x tile and the L1/attention weights land first
            w2s = wp.tile([128, D, 2, 2, 128], BF16)
            for d in range(D):
                nc.sync.dma_start(w2s[:, d], w2.ap()[:, d])

            NG = D // 4  # 16 groups of 4 d's; group dg holds d = 4*dg + j (j = row-group)

            def emit_attn(cidx, hps):
                    # ---- attention fusion + cross + g (per chunk, single psum slot) ----
                    hlo = atp.tile([D, BT], F32, tag="hlo")
                    nc.vector.tensor_copy(hlo[:], hps[0:D])
                    hsum = atp.tile([D, BT], F32, tag="hsum")
                    nc.vector.tensor_add(hsum[:], hlo[:], hps[D : 2 * D])
                    hcf = atp.tile([D, BT], F32, tag="hcf")
                    nc.scalar.activation(hcf[:], hsum[:], AF.Identity, bias=smalls["b3"][:])
                    hcb = atp.tile([D, BT], BF16, tag="hcb")
                    nc.vector.tensor_copy(hcb[:], hcf[:])

                    qk = []
                    for wname, bname in (("wq", "bq"), ("wk", "bk")):
                        qsb = atp.tile([128, 2, BT], F32, tag=wname + "sb")
                        for hc in range(2):
                            qp = psa.tile([128, BT], F32, tag="at")
                            nc.tensor.matmul(qp[:], smalls[wname][:, hc * 128 : (hc + 1) * 128],
                                             hcb[:], start=True, stop=True)
                            nc.scalar.activation(qsb[:, hc], qp[:], AF.Identity,
                                                 bias=smalls[bname][:, hc : hc + 1])
                        qk.append(qsb)
                    tsb = atp.tile([128, 2, BT], BF16, tag="tanh")
                    for hc in range(2):
                        pr = atp.tile([128, BT], F32, tag="prqk")
                        nc.vector.tensor_mul(pr[:], qk[0][:, hc], qk[1][:, hc])
                        nc.scalar.activation(tsb[:, hc], pr[:], AF.Tanh)
                    sp = psa.tile([D, BT], F32, tag="at")
                    for hc in range(2):
                        nc.tensor.matmul(sp[:], smalls["ws"][:, hc], tsb[:, hc],
                                         start=(hc == 0), stop=(hc == 1))
                    es = atp.tile([D, BT], F32, tag="es")
                    nc.scalar.activation(es[:], sp[:], AF.Exp, bias=smalls["bs"][:])
                    ph = atp.tile([D, BT], F32, tag="ph")
                    nc.vector.tensor_mul(ph[:], hcf[:], es[:])
                    sump = psa.tile([1, BT], F32, tag="at")
                    nc.tensor.matmul(sump[:], smalls["ones"][:], es[:], start=True, stop=True)
                    rec = atp.tile([1, BT], F32, tag="rec")
                    nc.vector.reciprocal(rec[:], sump[:])
                    nump = psa.tile([1, BT], F32, tag="at")
                    nc.tensor.matmul(nump[:], smalls["ones"][:], ph[:], start=True, stop=True)
                    wtd = atp.tile([1, BT], F32, tag="wtd")
                    nc.vector.tensor_mul(wtd[:], nump[:], rec[:])

                    # cross MLP
                    c1b = atp.tile([128, 2, BT], BF16, tag="c1")
                    for hc in range(2):
                        cp = psa.tile([128, BT], F32, tag="at")
                        nc.tensor.matmul(cp[:], smalls["wc1"][:, hc * 128 : (hc + 1) * 128],
                                         hcb[:], start=True, stop=True)
                        nc.scalar.activation(c1b[:, hc], cp[:], AF.Relu,
                                             bias=smalls["bc1"][:, hc : hc + 1])
                    crp = psa.tile([1, BT], F32, tag="at")
                    for kc in range(2):
                        nc.tensor.matmul(crp[:], smalls["wc2"][:, kc : kc + 1], c1b[:, kc],
                                         start=(kc == 0), stop=(kc == 1))
                    comb0 = atp.tile([1, BT], F32, tag="comb0")
                    nc.vector.tensor_add(comb0[:], wtd[:], crp[:])
                    combb = atp.tile([1, BT], BF16, tag="combb")
                    nc.vector.tensor_scalar_add(combb[:], comb0[:], smalls["bc2"][0:1, 0:1])

                    # g MLP: relu(comb*Wg1+bg1) @ Wg2 + bg2
                    gsb = atp.tile([128, 2, BT], BF16, tag="g")
                    for hc in range(2):
                        gp = psa.tile([128, BT], F32, tag="at")
                        nc.tensor.matmul(gp[:], smalls["wg1"][0:1, hc * 128 : (hc + 1) * 128],
                                         combb[:], start=True, stop=True)
                        nc.scalar.activation(gsb[:, hc], gp[:], AF.Relu,
                                             bias=smalls["bg1"][:, hc : hc + 1])
                    op = psa.tile([1, BT], F32, tag="at")
                    for kc in range(2):
                        nc.tensor.matmul(op[:], smalls["wg2"][:, kc : kc + 1], gsb[:, kc],
                                         start=(kc == 0), stop=(kc == 1))
                    of = atp.tile([1, BT], F32, tag="of")
                    nc.vector.tensor_scalar_add(of[:], op[:], smalls["bg2"][0:1, 0:1])
                    nc.sync.dma_start(yout.ap()[cidx : cidx + 1], of[:])

            prev = None
            import os as _os
            _rep = int(_os.environ.get("KERNEL_REPEAT", "1"))
            for c in [cc for _ in range(_rep) for cc in range(NCH)]:
                # chunk-local transposed x: partition 32j holds x[:, d] for d%4==j
                xtc = xp.tile([128, 16, BT], BF16, tag="xt")
                for g in range(4):
                    nc.sync.dma_start(xtc[32 * g : 32 * g + 2],
                                      xin.ap()[g, :, :, c * BT : (c + 1) * BT])
                a1sb_of = {}

                def emit_l1(dg, xtc=xtc, a1sb_of=a1sb_of):
                    # K=2 outer products (weight row + bias row vs x row + ones row);
                    # 4 per hc batch across PE row-groups -> concurrent on HW.
                    # Bias is folded into PSUM, so relu merges across d-pairs.
                    for p in range(2):
                        a1t = ap1.tile([128, 2, 2, BT], BF16, tag="a1")
                        a1sb_of[(dg, p)] = a1t
                    for hc in range(2):
                        a1ps = []
                        for p in range(2):
                            a1p = ps1.tile([128, 2, BT], F32, tag="a1p")
                            a1ps.append(a1p)
                        for j in range(4):
                            d = 4 * dg + j
                            nc.tensor.matmul(
                                a1ps[j // 2][:, j % 2],
                                w1s[32 * j : 32 * j + 2, d // 4, hc * 128 : (hc + 1) * 128],
                                xtc[32 * j : 32 * j + 2, d // 4],
                                start=True, stop=True, tile_position=(32 * j, 0),
                            )
                        for p in range(2):
                            nc.scalar.activation(a1sb_of[(dg, p)][:, :, hc],
                                                 a1ps[p][:], AF.Relu)

                emit_l1(0)
                emit_l1(1)
                if prev is not None:
                    emit_attn(*prev)
                    prev = None
                hps = psh.tile([128, BT], F32)
                for dg in range(NG):
                    if dg + 2 < NG:
                        emit_l1(dg + 2)
                    for j in range(4):
                        d = 4 * dg + j
                        a1sb = a1sb_of[(dg, j // 2)]
                        if j == 3:
                            a1sb_of.pop((dg, 0)); a1sb_of.pop((dg, 1))
                        a2sb = ap2.tile([128, 2, BT], BF16, tag="a2")
                        for kc in range(2):
                            a2p = ps2.tile([128, BT], F32, tag="a2p")
                            for hc in range(2):
                                nc.tensor.matmul(a2p[:], w2s[:, d, hc, kc],
                                                 a1sb[:, j % 2, hc],
                                                 start=(hc == 0), stop=(hc == 1))
                            nc.vector.tensor_scalar(a2sb[:, kc], a2p[:],
                                                    b2s[:, kc, d : d + 1], 0.0,
                                                    ALU.add, ALU.max)
                            # L3: kc halves target different PSUM col-groups (concurrent)
                            nc.tensor.matmul(hps[64 * kc : 64 * kc + 64],
                                             w3s[:, kc, d], a2sb[:, kc],
                                             start=(d == 0), stop=(d == D - 1),
                                             skip_group_check=True,
                                             tile_position=(0, 64 * kc))

                prev = (c, hps)
            if prev is not None:
                emit_attn(*prev)

    nc.compile()
    return nc


def _prep_weights(W1, b1, W2, b2, W3, b3, Wq, bq, Wk, bk, Ws, bs,
                  Wc1, bc1, Wc2, bc2, Wg1, bg1, Wg2, bg2):
    bf = ml_dtypes.bfloat16
    w1c = np.zeros((4, 2, 16, H), dtype=bf)
    for d in range(D):
        w1c[d % 4, 0, d // 4] = W1[d]
        w1c[d % 4, 1, d // 4] = b1[d]
    w2sb = np.ascontiguousarray(
        W2.reshape(D, 2, 128, 2, 128).transpose(2, 0, 1, 3, 4)).astype(bf)
    b2t = np.ascontiguousarray(b2.T.reshape(2, 128, D).transpose(1, 0, 2)).astype(np.float32)
    w3d = np.zeros((128, 2, D, D), dtype=bf)
    for d in range(D):
        for kc in range(2):
            w3d[:, kc, d, d] = W3[d, kc * 128 : (kc + 1) * 128].astype(bf)
    return {
        "w1c": w1c, "w2sb": w2sb, "b2t": b2t, "w3d": w3d,
        "b3c": b3[:, None].astype(np.float32),
        "wqt": np.ascontiguousarray(Wq.T).astype(bf),
        "bqt": np.ascontiguousarray(bq.reshape(2, 128).T).astype(np.float32),
        "wkt": np.ascontiguousarray(Wk.T).astype(bf),
        "bkt": np.ascontiguousarray(bk.reshape(2, 128).T).astype(np.float32),
        "wst": np.ascontiguousarray(Ws.T.reshape(2, 128, D).transpose(1, 0, 2)).astype(bf),
        "bsc": bs[:, None].astype(np.float32),
        "wc1t": np.ascontiguousarray(Wc1.T).astype(bf),
        "bc1t": np.ascontiguousarray(bc1.reshape(2, 128).T).astype(np.float32),
        "wc2c": np.ascontiguousarray(Wc2.reshape(2, 128).T).astype(bf),
        "bc2a": bc2.reshape(1, 1).astype(np.float32),
        "wg1r": Wg1[None, :].astype(bf),
        "bg1t": np.ascontiguousarray(bg1.reshape(2, 128).T).astype(np.float32),
        "wg2c": np.ascontiguousarray(Wg2.reshape(2, 128).T).astype(bf),
        "bg2a": bg2.reshape(1, 1).astype(np.float32),
        "ones64": np.ones((D, 1), dtype=np.float32),
    }


def kernel(x, W1, b1, W2, b2, W3, b3, Wq, bq, Wk, bk, Ws, bs,
           Wc1, bc1, Wc2, bc2, Wg1, bg1, Wg2, bg2):
    if "nc" not in _cache:
        _cache["nc"] = _build()
    nc = _cache["nc"]
    wmap = _prep_weights(W1, b1, W2, b2, W3, b3, Wq, bq, Wk, bk, Ws, bs,
                         Wc1, bc1, Wc2, bc2, Wg1, bg1, Wg2, bg2)
    bf = ml_dtypes.bfloat16
    in_maps = []
    for core in range(NCORES):
        xs = x[core * BLOC : (core + 1) * BLOC]          # [BLOC, D]
        xt = np.ascontiguousarray(xs.T)                  # [D, BLOC]
        xt32 = np.zeros((4, 2, 16, BLOC), dtype=bf)
        xt32[:, 1] = 1.0
        for d in range(D):
            xt32[d % 4, 0, d // 4] = xt[d]
        m = dict(wmap)
        m["xt32c"] = xt32
        in_maps.append(m)
    trace = bool(os.environ.get("KERNEL_TRACE"))
    res = bass_utils.run_bass_kernel_spmd(nc, in_maps, core_ids=list(range(NCORES)),
                                          trace=trace)
    _cache["last_exec_time_ns"] = res.exec_time_ns
    out = np.concatenate([res.results[c]["y"].reshape(BLOC, 1) for c in range(NCORES)], axis=0)
    return out.astype(np.float32)

